# revision 36
# baseline (speedup 1.0000x reference)
"""Trainium2 Bass kernel for GQA attention with RoPE (dense_transformer).

Model: B=2, T=2048, C=2048, H=16 query heads, KV=4 kv heads, D=128, causal.
Sharding: 8 cores = batch(2) x kv-group(4) tensor parallel. Each core computes
its batch's 4 query heads (one kv head), then a partial output projection over
its 512 head-dims; per-group ReduceScatters (4 cores per batch) sum the
partials. The host reassembles the stripes.

Final schedule (397us baseline -> ~368us):
 - Chunk-0 projection runs its six output chains (4q+k+v) in two waves of
   three, interleaved per contraction tile across PSUM banks, so the PE
   tracks the arriving (wqkv, x) tile pairs and is busy from ~1.5us instead
   of idling ~20us behind a serial weight load; wave-1 epilogues (evict +
   rope) overlap wave-2 matmuls.
 - RoPE reads the projection PSUM directly with cross-partition-base
   tensor_tensor ops (PSUM+SBUF operands may differ in base); no swap DMAs.
 - Softmax denominators accumulate on the vector engine (bf16 adds of the
   exp tiles); one all-ones [128,64]-stationary matmul per head reduces
   across partitions into a single PSUM bank (head0 rows 0:64, head1 rows
   64:128), one reciprocal covers both heads, and the normalize reads the PV
   PSUM with mismatched partition bases. Removes ~180 PE matmuls vs v2.
 - Groups run 0,1,3,2 so group 3's 2MB ReduceScatter overlaps group-2
   compute and only group 2's single 2MB RS is exposed at the tail.
 - Pinned 32KB scratch ReduceScatters rendezvous the CC streams during long
   RS-free stretches: cores drift 10-17us apart and every real collective
   otherwise pays that skew as trigger wait (measured 20 -> 40+us per RS).
 - RS output stores are deferred until after the last collective trigger so
   a store waiting on RS(i) never delays a later trigger on the gp queue.
"""

import os

os.environ.setdefault("MYCRO_LOCAL_CACHE", "1")

import numpy as np

B, T, C = 2, 2048, 2048
H, KV, D = 16, 4, 128
HL = H // KV          # 4 local query heads per core
NCORES = 8
P = 128
SCALE = 1.0 / float(np.sqrt(D))

NCC = C // P          # 16 contraction tiles
NCH = T // 512        # 4 t-chunks of 512
TQ = 512
NG = 4
NEG = -1e10

# last-processed group (g=2) RS: one rendezvous (skew dominates per-call
# latency, so fewer bigger calls win at the tail); groups 0,1,3 use [256, 256]
RS_SPLIT_LAST = [512]
OUT_ROWS = 512


def _emit(nc, tile, mybir, ExitStack):
    f32 = mybir.dt.float32
    bf16 = mybir.dt.bfloat16
    Exp = mybir.ActivationFunctionType.Exp
    Copy = mybir.ActivationFunctionType.Copy
    add = mybir.AluOpType.add
    mult = mybir.AluOpType.mult

    xt4 = nc.dram_tensor("xt4", [NCH * C, 512], bf16, kind="ExternalInput")
    wqkv = nc.dram_tensor("wqkv", [C, (HL + 2) * P], bf16, kind="ExternalInput")
    wot = nc.dram_tensor("wot", [HL * P, C], bf16, kind="ExternalInput")
    raq = nc.dram_tensor("raq", [P, T], bf16, kind="ExternalInput")
    rbq = nc.dram_tensor("rbq", [P, T], bf16, kind="ExternalInput")
    rak = nc.dram_tensor("rak", [P, T], bf16, kind="ExternalInput")
    rbk = nc.dram_tensor("rbk", [P, T], bf16, kind="ExternalInput")
    cstf = nc.dram_tensor("cstf", [P, P], f32, kind="ExternalInput")
    cstb = nc.dram_tensor("cstb", [P, 2 * P + 2], bf16, kind="ExternalInput")
    out = nc.dram_tensor("out", [OUT_ROWS, C], bf16, kind="ExternalOutput")

    te, ve, sc, gp, sy = nc.tensor, nc.vector, nc.scalar, nc.gpsimd, nc.sync

    with tile.TileContext(nc) as tc, ExitStack() as ctx:
        consts = ctx.enter_context(tc.tile_pool(name="consts", bufs=1))
        persist = ctx.enter_context(tc.tile_pool(name="persist", bufs=1))
        dram = ctx.enter_context(tc.tile_pool(name="dram", bufs=1, space="DRAM"))
        sbX = ctx.enter_context(tc.tile_pool(name="sbX", bufs=2))
        sbQK = ctx.enter_context(tc.tile_pool(name="sbQK", bufs=4))
        sbR = ctx.enter_context(tc.tile_pool(name="sbR", bufs=3))
        sbP = ctx.enter_context(tc.tile_pool(name="sbP", bufs=6))
        sbRC = ctx.enter_context(tc.tile_pool(name="sbRC", bufs=2))
        sbS = ctx.enter_context(tc.tile_pool(name="sbS", bufs=2))
        sbAT = ctx.enter_context(tc.tile_pool(name="sbAT", bufs=2))
        sbY = ctx.enter_context(tc.tile_pool(name="sbY", bufs=3))
        psF = ctx.enter_context(tc.tile_pool(name="psF", bufs=2, space="PSUM"))
        psS = ctx.enter_context(tc.tile_pool(name="psS", bufs=3, space="PSUM"))
        psA = ctx.enter_context(tc.tile_pool(name="psA", bufs=1, space="PSUM"))
        psB = ctx.enter_context(tc.tile_pool(name="psB", bufs=1, space="PSUM"))
        psD = ctx.enter_context(tc.tile_pool(name="psD", bufs=1, space="PSUM"))

        # constants come in via DMA (GpSimd-computed consts raced their
        # first-run consumers on cold ucode); sc queue — sy is reserved for
        # the chunk-0 x tiles that gate the first matmul
        ident = consts.tile([P, P], bf16, tag="ident")
        sc.dma_start(ident[:], cstb.ap()[:, 0:P])
        onesc = consts.tile([P, P], bf16, tag="onesc")
        sc.dma_start(onesc[:], cstb.ap()[:, P + 2:2 * P + 2])
        # scoresT layout [tk, tq]: keep where tq >= tk, else -1e10.
        triT = consts.tile([P, P], f32, tag="triT")
        sc.dma_start(triT[:], cstf.ap()[:, :])

        wqkvT = [persist.tile([P, (HL + 2) * P], bf16, tag=f"wqkvT{cc}",
                              name=f"wqkvT{cc}") for cc in range(NCC)]
        woTs = [persist.tile([P, C], bf16, tag=f"woT{h}", name=f"woT{h}")
                for h in range(HL)]
        tabs = {}
        for nm in ("aq", "bq", "ak", "bk"):
            tabs[nm] = persist.tile([P, T], bf16, tag=f"tab_{nm}",
                                    name=f"tab_{nm}")
        qrT = [persist.tile([P, T], bf16, tag=f"qrT{h}", name=f"qrT{h}")
               for h in range(HL)]
        krT = persist.tile([P, T], bf16, tag="krT")
        vnat = persist.tile([P, T], bf16, tag="vnat")

        y_dram = [dram.tile([TQ, C], bf16, tag=f"ydram{g}", name=f"ydram{g}")
                  for g in range(NG)]
        # 7 RS outputs: groups 0,1,3 -> 2x64 rows; group 2 -> one 128
        rs_rows = [64, 64, 64, 64, 64, 64, 128]
        rs_out = [dram.tile([r, C], bf16, tag=f"rsout{i}", name=f"rsout{i}")
                  for i, r in enumerate(rs_rows)]

        # ---- startup DMAs: x chunk0 on sy, wqkv on gp, rope tables on sc,
        # wo on gp afterwards. The interleaved chunk-0 proj consumes pair cc
        # at ~1.3us each, above the per-queue delivery rate, so PE never
        # stalls after the first pair.
        xts0 = []
        for cc in range(NCC):
            xt = sbX.tile([P, 512], bf16, tag=f"xT{cc}", name=f"xT{cc}")
            xts0.append(xt)
        for cc in range(NCC):
            gp.dma_start(wqkvT[cc][:], wqkv.ap()[cc * P:(cc + 1) * P, :])
            sy.dma_start(xts0[cc][:], xt4.ap()[cc * P:cc * P + P, :])
        for nm, src in (("aq", raq), ("bq", rbq), ("ak", rak), ("bk", rbk)):
            sc.dma_start(tabs[nm][:], src.ap()[:, :])
        for h in range(HL):
            gp.dma_start(woTs[h][:], wot.ap()[h * P:(h + 1) * P, :])

        def rope(dst, sl, ps, qs, ta, tb):
            # dst[:,sl] = qs * ta + cross_half(ps) * tb ; ps is the PSUM copy
            # of qs (f32). PSUM+SBUF tensor_tensor allows differing partition
            # bases, so the half swap needs no data movement.
            t1 = sbR.tile([P, 512], bf16, tag="ropet1", name="ropet1")
            t2 = sbR.tile([P, 512], bf16, tag="ropet2", name="ropet2")
            ve.tensor_mul(t1[:], qs[:], ta[:, sl])
            with nc.allow_low_precision(reason="rope bf16"):
                ve.tensor_tensor(t2[0:64, :], ps[64:P, :], tb[0:64, sl], mult)
                ve.tensor_tensor(t2[64:P, :], ps[0:64, :], tb[64:P, sl], mult)
            ve.tensor_add(dst[:, sl], t1[:], t2[:])

        def proj_epilogue(ch, kind, ps):
            # kind: 0..HL-1 = q head, HL = k, HL+1 = v
            sl = slice(ch * 512, (ch + 1) * 512)
            qs = sbQK.tile([P, 512], bf16, tag="qkev", name="qkev")
            sc.activation(qs[:], ps[:], Copy)
            if kind < HL:
                rope(qrT[kind], sl, ps, qs, tabs["aq"], tabs["bq"])
            elif kind == HL:
                rope(krT, sl, ps, qs, tabs["ak"], tabs["bk"])
            else:
                pv = psF.tile([P, 512], bf16, tag="fat", name="fat")
                for i in range(4):
                    te.transpose(pv[:, i * P:(i + 1) * P],
                                 qs[:, i * P:(i + 1) * P], ident[:])
                sc.activation(vnat[:, sl], pv[:], Copy)

        def proj0_interleaved():
            # two waves of three concurrent accumulation chains, interleaved
            # per contraction tile, so the matmuls track the arriving
            # (wqkv, x) pairs in cc order AND wave-1 epilogues (eviction +
            # rope on ACT/DVE) overlap wave-2 matmuls instead of exposing a
            # serial epilogue burst at the end of the chunk
            banks = [psF.tile([P, 512], f32, tag="fat", name="fat"),
                     psF.tile([P, 512], f32, tag="fat", name="fat"),
                     psS.tile([P, 512], f32, tag="score", name="score"),
                     psS.tile([P, 512], f32, tag="score", name="score"),
                     psS.tile([P, 512], f32, tag="score", name="score"),
                     psA.tile([P, 512], f32, tag="paA", name="paA")]
            for wave in range(2):
                ks = (0, 1, 2) if wave == 0 else (3, 4, 5)
                for cc in range(NCC):
                    for k in ks:
                        te.matmul(banks[k][:],
                                  wqkvT[cc][:, k * P:(k + 1) * P],
                                  xts0[cc][:],
                                  start=(cc == 0), stop=(cc == NCC - 1))
                for k in ks:
                    proj_epilogue(0, k, banks[k])

        def proj(ch):
            xts = []
            for cc in range(NCC):
                xt = sbX.tile([P, 512], bf16, tag=f"xT{cc}", name=f"xT{cc}")
                (sy if cc % 2 == 0 else sc).dma_start(
                    xt[:], xt4.ap()[ch * C + cc * P:ch * C + (cc + 1) * P, :])
                xts.append(xt)
            for k in range(HL + 2):
                ps = psF.tile([P, 512], f32, tag="fat", name="fat")
                for cc in range(NCC):
                    te.matmul(ps[:], wqkvT[cc][:, k * P:(k + 1) * P], xts[cc][:],
                              start=(cc == 0), stop=(cc == NCC - 1))
                proj_epilogue(ch, k, ps)

        def emit_scores(gq, kb, hs):
            j = kb - 4 * gq
            w0 = max(j, 0) * P
            probs = []
            for h in hs:
                st = psS.tile([P, TQ], f32, tag="score", name="score")
                te.matmul(
                    st[:, w0:TQ],
                    krT[:, kb * P:(kb + 1) * P],
                    qrT[h][:, gq * TQ + w0:(gq + 1) * TQ],
                    start=True, stop=True,
                )
                if j >= 0:
                    ve.tensor_tensor(
                        st[:, w0:w0 + P], st[:, w0:w0 + P], triT[:], add)
                pb = sbP.tile([P, TQ], bf16, tag="probs", name="probs")
                sc.activation(pb[:, w0:TQ], st[:, w0:TQ], Exp)
                probs.append(pb)
            return probs, w0

        def emit_accum(kb, kbmax, w0, probs, pa, S):
            # prob-sums accumulate on the vector engine (bf16); the final
            # partition-reduce happens once per head via a ones matmul
            for i in range(2):
                if kb == 0:
                    ve.tensor_copy(S[i][:], probs[i][:])
                else:
                    ve.tensor_tensor(S[i][:, w0:TQ], S[i][:, w0:TQ],
                                     probs[i][:, w0:TQ], add)
            for i in range(2):
                te.matmul(
                    pa[i][:, w0:TQ], vnat[:, kb * P:(kb + 1) * P],
                    probs[i][:, w0:TQ],
                    start=(kb == 0), stop=(kb == kbmax - 1),
                )

        attn_cur = {}

        # tiny scratch collective: rendezvous the CC streams periodically.
        # Cores drift ~10-17us apart over unsynchronized stretches, and every
        # real RS then pays that skew as trigger-wait; a 32KB sync bounds it.
        # The input is pinned to a just-produced tile via a 1-row DMA so the
        # scheduler cannot hoist the sync to t=0 (it did).
        warm_in = dram.tile([8, C], bf16, tag="warm_in", name="warm_in")
        warm_out = dram.tile([2, C], bf16, tag="warm_out", name="warm_out")

        def cc_warm(dep_ap):
            sy.dma_start(warm_in[0:1, 0:P], dep_ap)
            gp.collective_compute(
                "ReduceScatter", mybir.AluOpType.add,
                replica_groups=[[0, 1, 2, 3], [4, 5, 6, 7]],
                ins=[warm_in.opt()],
                outs=[warm_out.opt()],
            )

        def attn(gq, mid=False):
            kbmax = 4 * (gq + 1)
            for hp in range(HL // 2):
                if hp == 1 and mid:
                    cc_warm(attn_cur[1][0:1, 0:P])
                hs = (2 * hp, 2 * hp + 1)
                pa = [psA.tile([P, TQ], f32, tag="paA", name="paA"),
                      psB.tile([P, TQ], f32, tag="paB", name="paB")]
                S = [sbS.tile([P, TQ], bf16, tag="S0", name="S0"),
                     sbS.tile([P, TQ], bf16, tag="S1", name="S1")]
                prev = None
                for kb in range(kbmax):
                    cur = (kb, *emit_scores(gq, kb, hs))
                    if prev is not None:
                        pkb, pprobs, pw0 = prev
                        emit_accum(pkb, kbmax, pw0, pprobs, pa, S)
                    prev = cur
                pkb, pprobs, pw0 = prev
                emit_accum(pkb, kbmax, pw0, pprobs, pa, S)

                # per-head denominator: one partition-reduce matmul from S,
                # broadcast over 64 partitions per head
                psums = psD.tile([P, TQ], f32, tag="psums", name="psums")
                for i in range(2):
                    te.matmul(psums[64 * i:64 * (i + 1), :], onesc[:, 0:64],
                              S[i][:], start=True, stop=True)
                recf = sbRC.tile([P, TQ], f32, tag="recf", name="recf")
                ve.reciprocal_approx_fast(recf[:], psums[:])
                for i, h in enumerate(hs):
                    at = sbAT.tile([P, TQ], bf16, tag=f"attnT{h}",
                                   name=f"attnT{h}")
                    rb = recf[64 * i:64 * (i + 1), :]
                    with nc.allow_low_precision(reason="softmax norm bf16"):
                        ve.tensor_tensor(at[0:64, :], pa[i][0:64, :], rb, mult)
                        ve.tensor_tensor(at[64:P, :], pa[i][64:P, :], rb, mult)
                    attn_cur[h] = at

        def outproj(gq):
            for tb in range(4):
                ysb = sbY.tile([P, C], bf16, tag="ysb", name="ysb")
                for cc4 in range(4):
                    py = psF.tile([P, 512], f32, tag="fat", name="fat")
                    for h in range(HL):
                        te.matmul(
                            py[:],
                            attn_cur[h][:, tb * P:(tb + 1) * P],
                            woTs[h][:, cc4 * 512:(cc4 + 1) * 512],
                            start=(h == 0), stop=(h == HL - 1),
                        )
                    ve.tensor_copy(ysb[:, cc4 * 512:(cc4 + 1) * 512], py[:])
                sy.dma_start(y_dram[gq][tb * P:(tb + 1) * P, :], ysb[:])

        rs_state = [0]
        deferred_stores = []

        def emit_rs(gq, splits):
            row = 0
            for nrows in splits:
                i = rs_state[0]
                rs_state[0] += 1
                gp.collective_compute(
                    "ReduceScatter", mybir.AluOpType.add,
                    replica_groups=[[0, 1, 2, 3], [4, 5, 6, 7]],
                    ins=[y_dram[gq][row:row + nrows, :].opt()],
                    outs=[rs_out[i].opt()],
                )
                # output stores are deferred to after the last trigger so a
                # store waiting on RS(i) never delays a later collective's
                # trigger on the gp queue
                deferred_stores.append(i)
                row += nrows

        # ---- pipelined schedule: groups 0,1,3,2 ---------------------------
        proj0_interleaved()
        proj(1)
        attn(0)
        outproj(0)
        emit_rs(0, [256, 256])
        attn(1)
        outproj(1)
        emit_rs(1, [256, 256])
        proj(2)
        cc_warm(vnat[0:1, 2 * 512:2 * 512 + P])
        proj(3)
        cc_warm(vnat[0:1, 3 * 512:3 * 512 + P])
        attn(3, mid=True)
        outproj(3)
        emit_rs(3, [256, 256])
        attn(2)
        outproj(2)
        emit_rs(2, RS_SPLIT_LAST)
        for i in deferred_stores:
            gp.dma_start(
                out.ap()[sum(rs_rows[:i]):sum(rs_rows[:i + 1]), :],
                rs_out[i][:])

    return nc


_PROGRAM = None


def _get_program():
    global _PROGRAM
    if _PROGRAM is None:
        from contextlib import ExitStack
        import concourse.tile as tile
        from concourse import bacc, mybir

        nc = bacc.Bacc("TRN2", target_bir_lowering=False, debug=False,
                       num_devices=NCORES)
        _emit(nc, tile, mybir, ExitStack)
        nc.compile()
        _PROGRAM = nc
    return _PROGRAM


def _bf16(a):
    from ml_dtypes import bfloat16
    return np.asarray(a, np.float32).astype(bfloat16)


def _perm_eo(w):
    """Per 128-row block: rows -> [even rows (64), odd rows (64)]."""
    n = w.shape[0] // P
    w = w.reshape(n, 64, 2, w.shape[-1])
    return np.concatenate([w[:, :, 0, :], w[:, :, 1, :]], axis=1).reshape(
        n * P, -1)


def make_in_maps(x, wq, wk, wv, wo, freqs_cos, freqs_sin):
    x = np.asarray(x, np.float32)
    cos = np.asarray(freqs_cos, np.float32)
    sin = np.asarray(freqs_sin, np.float32)

    cosT = cos.T                      # [64, T]
    sinT = sin.T
    ak = _bf16(np.ascontiguousarray(np.vstack([cosT, cosT])))
    bk = _bf16(np.ascontiguousarray(np.vstack([-sinT, sinT])))
    aq = _bf16(SCALE * np.vstack([cosT, cosT]))
    bq = _bf16(SCALE * np.vstack([-sinT, sinT]))

    xt4s = []
    for b in range(B):
        xt = _bf16(x[b]).T            # [C, T]
        xt4 = np.ascontiguousarray(
            xt.reshape(C, NCH, 512).transpose(1, 0, 2)).reshape(NCH * C, 512)
        xt4s.append(xt4)

    # device constants: causal mask (scoresT layout), identity, ones
    tri = np.where(np.arange(P)[None, :] >= np.arange(P)[:, None],
                   np.float32(0.0), np.float32(NEG)).astype(np.float32)
    cstb = np.zeros((P, 2 * P + 2), np.float32)
    cstb[:, 0:P] = np.eye(P, dtype=np.float32)
    cstb[:, P:] = 1.0
    cstb = _bf16(cstb)

    in_maps = []
    for core in range(NCORES):
        b, g = core // 4, core % 4
        wq_g = _perm_eo(np.asarray(wq[g * HL * D:(g + 1) * HL * D], np.float32))
        wk_g = _perm_eo(np.asarray(wk[g * D:(g + 1) * D], np.float32))
        wv_g = np.asarray(wv[g * D:(g + 1) * D], np.float32)
        wqkv_g = _bf16(np.ascontiguousarray(
            np.concatenate([wq_g.T, wk_g.T, wv_g.T], axis=1)))
        wot_g = _bf16(np.ascontiguousarray(
            np.asarray(wo, np.float32)[:, g * HL * D:(g + 1) * HL * D].T))
        in_maps.append({
            "xt4": xt4s[b],
            "wqkv": wqkv_g,
            "wot": wot_g,
            "raq": aq, "rbq": bq, "rak": ak, "rbk": bk,
            "cstf": tri, "cstb": cstb,
        })
    return in_maps


def kernel(x, wq, wk, wv, wo, freqs_cos, freqs_sin, mask=None):
    from concourse.bass_utils import run_bass_kernel_spmd

    nc = _get_program()
    in_maps = make_in_maps(x, wq, wk, wv, wo, freqs_cos, freqs_sin)
    res = run_bass_kernel_spmd(nc, in_maps, core_ids=list(range(NCORES)))
    outp = np.empty((B, T, C), np.float32)
    # piece row blocks, in RS emission order (groups 0,1,3,2):
    # (piece_row_start, nrows, global query base for core stripe 0)
    blocks = [(0, 64, 0), (64, 64, 256),        # g0 halves
              (128, 64, 512), (192, 64, 768),   # g1 halves
              (256, 64, 1536), (320, 64, 1792),  # g3 halves
              (384, 128, 1024)]                  # g2 single RS
    for b in range(B):
        for r in range(4):
            piece = np.asarray(res.results[4 * b + r]["out"],
                               dtype=np.float32)  # [512, C]
            for prow, nrows, qbase in blocks:
                dst = qbase + nrows * r
                outp[b, dst:dst + nrows] = piece[prow:prow + nrows]
    return outp


# revision 37
# speedup vs baseline: 1.0840x; 1.0840x over previous
"""Trainium2 Bass kernel for GQA attention with RoPE (dense_transformer).

Model: B=2, T=2048, C=2048, H=16 query heads, KV=4 kv heads, D=128, causal.
Sharding: 8 cores = batch(2) x kv-group(4) tensor parallel. Each core computes
its batch's 4 query heads (one kv head), then a partial output projection over
its 512 head-dims; per-group ReduceScatters (4 cores per batch) sum the
partials. The host reassembles the stripes.

Final schedule (397us baseline -> ~368us):
 - Chunk-0 projection runs its six output chains (4q+k+v) in two waves of
   three, interleaved per contraction tile across PSUM banks, so the PE
   tracks the arriving (wqkv, x) tile pairs and is busy from ~1.5us instead
   of idling ~20us behind a serial weight load; wave-1 epilogues (evict +
   rope) overlap wave-2 matmuls.
 - RoPE reads the projection PSUM directly with cross-partition-base
   tensor_tensor ops (PSUM+SBUF operands may differ in base); no swap DMAs.
 - Softmax denominators accumulate on the vector engine (bf16 adds of the
   exp tiles); one all-ones [128,64]-stationary matmul per head reduces
   across partitions into a single PSUM bank (head0 rows 0:64, head1 rows
   64:128), one reciprocal covers both heads, and the normalize reads the PV
   PSUM with mismatched partition bases. Removes ~180 PE matmuls vs v2.
 - Groups run 0,1,3,2 so group 3's 2MB ReduceScatter overlaps group-2
   compute and only group 2's single 2MB RS is exposed at the tail.
 - Pinned 32KB scratch ReduceScatters rendezvous the CC streams during long
   RS-free stretches: cores drift 10-17us apart and every real collective
   otherwise pays that skew as trigger wait (measured 20 -> 40+us per RS).
 - RS output stores are deferred until after the last collective trigger so
   a store waiting on RS(i) never delays a later trigger on the gp queue.
"""

import os

os.environ.setdefault("MYCRO_LOCAL_CACHE", "1")

import numpy as np

B, T, C = 2, 2048, 2048
H, KV, D = 16, 4, 128
HL = H // KV          # 4 local query heads per core
NCORES = 8
P = 128
SCALE = 1.0 / float(np.sqrt(D))

NCC = C // P          # 16 contraction tiles
NCH = T // 512        # 4 t-chunks of 512
TQ = 512
NG = 4
NEG = -1e10

# last-processed group (g=2) RS: one rendezvous (skew dominates per-call
# latency, so fewer bigger calls win at the tail); groups 0,1,3 use [256, 256]
RS_SPLIT_LAST = [512]
OUT_ROWS = 512


def _emit(nc, tile, mybir, ExitStack):
    f32 = mybir.dt.float32
    bf16 = mybir.dt.bfloat16
    Exp = mybir.ActivationFunctionType.Exp
    Copy = mybir.ActivationFunctionType.Copy
    add = mybir.AluOpType.add
    mult = mybir.AluOpType.mult

    xt4 = nc.dram_tensor("xt4", [NCH * C, 512], bf16, kind="ExternalInput")
    wqkv = nc.dram_tensor("wqkv", [C, (HL + 2) * P], bf16, kind="ExternalInput")
    wot = nc.dram_tensor("wot", [HL * P, C], bf16, kind="ExternalInput")
    raq = nc.dram_tensor("raq", [P, T], bf16, kind="ExternalInput")
    rbq = nc.dram_tensor("rbq", [P, T], bf16, kind="ExternalInput")
    rak = nc.dram_tensor("rak", [P, T], bf16, kind="ExternalInput")
    rbk = nc.dram_tensor("rbk", [P, T], bf16, kind="ExternalInput")
    cstf = nc.dram_tensor("cstf", [P, P], f32, kind="ExternalInput")
    cstb = nc.dram_tensor("cstb", [P, 2 * P + 2], bf16, kind="ExternalInput")
    out = nc.dram_tensor("out", [OUT_ROWS, C], bf16, kind="ExternalOutput")

    te, ve, sc, gp, sy = nc.tensor, nc.vector, nc.scalar, nc.gpsimd, nc.sync

    with tile.TileContext(nc) as tc, ExitStack() as ctx:
        consts = ctx.enter_context(tc.tile_pool(name="consts", bufs=1))
        persist = ctx.enter_context(tc.tile_pool(name="persist", bufs=1))
        dram = ctx.enter_context(tc.tile_pool(name="dram", bufs=1, space="DRAM"))
        sbX = ctx.enter_context(tc.tile_pool(name="sbX", bufs=2))
        sbQK = ctx.enter_context(tc.tile_pool(name="sbQK", bufs=4))
        sbR = ctx.enter_context(tc.tile_pool(name="sbR", bufs=3))
        sbP = ctx.enter_context(tc.tile_pool(name="sbP", bufs=6))
        sbRC = ctx.enter_context(tc.tile_pool(name="sbRC", bufs=2))
        sbS = ctx.enter_context(tc.tile_pool(name="sbS", bufs=2))
        sbAT = ctx.enter_context(tc.tile_pool(name="sbAT", bufs=2))
        sbY = ctx.enter_context(tc.tile_pool(name="sbY", bufs=3))
        psF = ctx.enter_context(tc.tile_pool(name="psF", bufs=2, space="PSUM"))
        psS = ctx.enter_context(tc.tile_pool(name="psS", bufs=3, space="PSUM"))
        psA = ctx.enter_context(tc.tile_pool(name="psA", bufs=1, space="PSUM"))
        psB = ctx.enter_context(tc.tile_pool(name="psB", bufs=1, space="PSUM"))
        psD = ctx.enter_context(tc.tile_pool(name="psD", bufs=1, space="PSUM"))

        # constants come in via DMA (GpSimd-computed consts raced their
        # first-run consumers on cold ucode); sc queue — sy is reserved for
        # the chunk-0 x tiles that gate the first matmul
        ident = consts.tile([P, P], bf16, tag="ident")
        sc.dma_start(ident[:], cstb.ap()[:, 0:P])
        onesc = consts.tile([P, P], bf16, tag="onesc")
        sc.dma_start(onesc[:], cstb.ap()[:, P + 2:2 * P + 2])
        # scoresT layout [tk, tq]: keep where tq >= tk, else -1e10.
        triT = consts.tile([P, P], f32, tag="triT")
        sc.dma_start(triT[:], cstf.ap()[:, :])

        wqkvT = [persist.tile([P, (HL + 2) * P], bf16, tag=f"wqkvT{cc}",
                              name=f"wqkvT{cc}") for cc in range(NCC)]
        woTs = [persist.tile([P, C], bf16, tag=f"woT{h}", name=f"woT{h}")
                for h in range(HL)]
        tabs = {}
        for nm in ("aq", "bq", "ak", "bk"):
            tabs[nm] = persist.tile([P, T], bf16, tag=f"tab_{nm}",
                                    name=f"tab_{nm}")
        qrT = [persist.tile([P, T], bf16, tag=f"qrT{h}", name=f"qrT{h}")
               for h in range(HL)]
        krT = persist.tile([P, T], bf16, tag="krT")
        vnat = persist.tile([P, T], bf16, tag="vnat")

        y_dram = [dram.tile([TQ, C], bf16, tag=f"ydram{g}", name=f"ydram{g}")
                  for g in range(NG)]
        # 7 RS outputs: groups 0,1,3 -> 2x64 rows; group 2 -> one 128
        rs_rows = [64, 64, 64, 64, 64, 64, 128]
        rs_out = [dram.tile([r, C], bf16, tag=f"rsout{i}", name=f"rsout{i}")
                  for i, r in enumerate(rs_rows)]

        # ---- startup DMAs: x chunk0 on sy, wqkv on gp, rope tables on sc,
        # wo on gp afterwards. The interleaved chunk-0 proj consumes pair cc
        # at ~1.3us each, above the per-queue delivery rate, so PE never
        # stalls after the first pair.
        xts0 = []
        for cc in range(NCC):
            xt = sbX.tile([P, 512], bf16, tag=f"xT{cc}", name=f"xT{cc}")
            xts0.append(xt)
        for cc in range(NCC):
            gp.dma_start(wqkvT[cc][:], wqkv.ap()[cc * P:(cc + 1) * P, :])
            sy.dma_start(xts0[cc][:], xt4.ap()[cc * P:cc * P + P, :])
        for nm, src in (("aq", raq), ("bq", rbq), ("ak", rak), ("bk", rbk)):
            sc.dma_start(tabs[nm][:], src.ap()[:, :])
        for h in range(HL):
            gp.dma_start(woTs[h][:], wot.ap()[h * P:(h + 1) * P, :])

        def rope(dst, sl, ps, qs, ta, tb):
            # dst[:,sl] = qs * ta + cross_half(ps) * tb ; ps is the PSUM copy
            # of qs (f32). PSUM+SBUF tensor_tensor allows differing partition
            # bases, so the half swap needs no data movement.
            t1 = sbR.tile([P, 512], bf16, tag="ropet1", name="ropet1")
            t2 = sbR.tile([P, 512], bf16, tag="ropet2", name="ropet2")
            ve.tensor_mul(t1[:], qs[:], ta[:, sl])
            with nc.allow_low_precision(reason="rope bf16"):
                ve.tensor_tensor(t2[0:64, :], ps[64:P, :], tb[0:64, sl], mult)
                ve.tensor_tensor(t2[64:P, :], ps[0:64, :], tb[64:P, sl], mult)
            ve.tensor_add(dst[:, sl], t1[:], t2[:])

        def proj_epilogue(ch, kind, ps):
            # kind: 0..HL-1 = q head, HL = k, HL+1 = v
            sl = slice(ch * 512, (ch + 1) * 512)
            qs = sbQK.tile([P, 512], bf16, tag="qkev", name="qkev")
            sc.activation(qs[:], ps[:], Copy)
            if kind < HL:
                rope(qrT[kind], sl, ps, qs, tabs["aq"], tabs["bq"])
            elif kind == HL:
                rope(krT, sl, ps, qs, tabs["ak"], tabs["bk"])
            else:
                pv = psF.tile([P, 512], bf16, tag="fat", name="fat")
                for i in range(4):
                    te.transpose(pv[:, i * P:(i + 1) * P],
                                 qs[:, i * P:(i + 1) * P], ident[:])
                sc.activation(vnat[:, sl], pv[:], Copy)

        def proj0_interleaved():
            # two waves of three concurrent accumulation chains, interleaved
            # per contraction tile, so the matmuls track the arriving
            # (wqkv, x) pairs in cc order AND wave-1 epilogues (eviction +
            # rope on ACT/DVE) overlap wave-2 matmuls instead of exposing a
            # serial epilogue burst at the end of the chunk
            banks = [psF.tile([P, 512], f32, tag="fat", name="fat"),
                     psF.tile([P, 512], f32, tag="fat", name="fat"),
                     psS.tile([P, 512], f32, tag="score", name="score"),
                     psS.tile([P, 512], f32, tag="score", name="score"),
                     psS.tile([P, 512], f32, tag="score", name="score"),
                     psA.tile([P, 512], f32, tag="paA", name="paA")]
            for wave in range(2):
                ks = (0, 1, 2) if wave == 0 else (3, 4, 5)
                for cc in range(NCC):
                    for k in ks:
                        te.matmul(banks[k][:],
                                  wqkvT[cc][:, k * P:(k + 1) * P],
                                  xts0[cc][:],
                                  start=(cc == 0), stop=(cc == NCC - 1))
                for k in ks:
                    proj_epilogue(0, k, banks[k])

        def proj(ch):
            xts = []
            for cc in range(NCC):
                xt = sbX.tile([P, 512], bf16, tag=f"xT{cc}", name=f"xT{cc}")
                (sy if cc % 2 == 0 else sc).dma_start(
                    xt[:], xt4.ap()[ch * C + cc * P:ch * C + (cc + 1) * P, :])
                xts.append(xt)
            for k in range(HL + 2):
                ps = psF.tile([P, 512], f32, tag="fat", name="fat")
                for cc in range(NCC):
                    te.matmul(ps[:], wqkvT[cc][:, k * P:(k + 1) * P], xts[cc][:],
                              start=(cc == 0), stop=(cc == NCC - 1))
                proj_epilogue(ch, k, ps)

        def emit_scores(gq, kb, hs):
            j = kb - 4 * gq
            w0 = max(j, 0) * P
            probs = []
            for h in hs:
                st = psS.tile([P, TQ], f32, tag="score", name="score")
                te.matmul(
                    st[:, w0:TQ],
                    krT[:, kb * P:(kb + 1) * P],
                    qrT[h][:, gq * TQ + w0:(gq + 1) * TQ],
                    start=True, stop=True,
                )
                if j >= 0:
                    ve.tensor_tensor(
                        st[:, w0:w0 + P], st[:, w0:w0 + P], triT[:], add)
                pb = sbP.tile([P, TQ], bf16, tag="probs", name="probs")
                sc.activation(pb[:, w0:TQ], st[:, w0:TQ], Exp)
                probs.append(pb)
            return probs, w0

        def emit_accum(kb, kbmax, w0, probs, pa, S):
            # prob-sums accumulate on the vector engine (bf16); the final
            # partition-reduce happens once per head via a ones matmul
            for i in range(2):
                if kb == 0:
                    ve.tensor_copy(S[i][:], probs[i][:])
                else:
                    ve.tensor_tensor(S[i][:, w0:TQ], S[i][:, w0:TQ],
                                     probs[i][:, w0:TQ], add)
            for i in range(2):
                te.matmul(
                    pa[i][:, w0:TQ], vnat[:, kb * P:(kb + 1) * P],
                    probs[i][:, w0:TQ],
                    start=(kb == 0), stop=(kb == kbmax - 1),
                )

        attn_cur = {}

        # tiny scratch collective: rendezvous the CC streams periodically.
        # Cores drift ~10-17us apart over unsynchronized stretches, and every
        # real RS then pays that skew as trigger-wait; a 32KB sync bounds it.
        # The input is pinned to a just-produced tile via a 1-row DMA so the
        # scheduler cannot hoist the sync to t=0 (it did).
        warm_in = dram.tile([8, C], bf16, tag="warm_in", name="warm_in")
        warm_out = dram.tile([2, C], bf16, tag="warm_out", name="warm_out")

        def cc_warm(dep_ap):
            sy.dma_start(warm_in[0:1, 0:P], dep_ap)
            gp.collective_compute(
                "ReduceScatter", mybir.AluOpType.add,
                replica_groups=[[0, 1, 2, 3], [4, 5, 6, 7]],
                ins=[warm_in.opt()],
                outs=[warm_out.opt()],
            )

        def attn(gq, mid=False):
            kbmax = 4 * (gq + 1)
            for hp in range(HL // 2):
                if hp == 1 and mid:
                    cc_warm(attn_cur[1][0:1, 0:P])
                hs = (2 * hp, 2 * hp + 1)
                pa = [psA.tile([P, TQ], f32, tag="paA", name="paA"),
                      psB.tile([P, TQ], f32, tag="paB", name="paB")]
                S = [sbS.tile([P, TQ], bf16, tag="S0", name="S0"),
                     sbS.tile([P, TQ], bf16, tag="S1", name="S1")]
                prev = None
                for kb in range(kbmax):
                    cur = (kb, *emit_scores(gq, kb, hs))
                    if prev is not None:
                        pkb, pprobs, pw0 = prev
                        emit_accum(pkb, kbmax, pw0, pprobs, pa, S)
                    prev = cur
                pkb, pprobs, pw0 = prev
                emit_accum(pkb, kbmax, pw0, pprobs, pa, S)

                # per-head denominator: one partition-reduce matmul from S,
                # broadcast over 64 partitions per head
                psums = psD.tile([P, TQ], f32, tag="psums", name="psums")
                for i in range(2):
                    te.matmul(psums[64 * i:64 * (i + 1), :], onesc[:, 0:64],
                              S[i][:], start=True, stop=True)
                recf = sbRC.tile([P, TQ], f32, tag="recf", name="recf")
                ve.reciprocal_approx_fast(recf[:], psums[:])
                for i, h in enumerate(hs):
                    at = sbAT.tile([P, TQ], bf16, tag=f"attnT{h}",
                                   name=f"attnT{h}")
                    rb = recf[64 * i:64 * (i + 1), :]
                    with nc.allow_low_precision(reason="softmax norm bf16"):
                        ve.tensor_tensor(at[0:64, :], pa[i][0:64, :], rb, mult)
                        ve.tensor_tensor(at[64:P, :], pa[i][64:P, :], rb, mult)
                    attn_cur[h] = at

        def outproj(gq):
            for tb in range(4):
                ysb = sbY.tile([P, C], bf16, tag="ysb", name="ysb")
                for cc4 in range(4):
                    py = psF.tile([P, 512], f32, tag="fat", name="fat")
                    for h in range(HL):
                        te.matmul(
                            py[:],
                            attn_cur[h][:, tb * P:(tb + 1) * P],
                            woTs[h][:, cc4 * 512:(cc4 + 1) * 512],
                            start=(h == 0), stop=(h == HL - 1),
                        )
                    ve.tensor_copy(ysb[:, cc4 * 512:(cc4 + 1) * 512], py[:])
                sy.dma_start(y_dram[gq][tb * P:(tb + 1) * P, :], ysb[:])

        rs_state = [0]
        deferred_stores = []

        def emit_rs(gq, splits):
            row = 0
            for nrows in splits:
                i = rs_state[0]
                rs_state[0] += 1
                gp.collective_compute(
                    "ReduceScatter", mybir.AluOpType.add,
                    replica_groups=[[0, 1, 2, 3], [4, 5, 6, 7]],
                    ins=[y_dram[gq][row:row + nrows, :].opt()],
                    outs=[rs_out[i].opt()],
                )
                # output stores are deferred to after the last trigger so a
                # store waiting on RS(i) never delays a later collective's
                # trigger on the gp queue
                deferred_stores.append(i)
                row += nrows

        # ---- pipelined schedule: groups 0,1,3,2 ---------------------------
        proj0_interleaved()
        proj(1)
        attn(0)
        outproj(0)
        emit_rs(0, [256, 256])
        attn(1, mid=True)
        outproj(1)
        emit_rs(1, [256, 256])
        proj(2)
        cc_warm(vnat[0:1, 2 * 512:2 * 512 + P])
        proj(3)
        cc_warm(vnat[0:1, 3 * 512:3 * 512 + P])
        attn(3, mid=True)
        outproj(3)
        emit_rs(3, [256, 256])
        attn(2)
        # pre-sync the CC streams while outproj(2) computes, so the final
        # 2MB ReduceScatter pays transfer time only, not core skew
        cc_warm(attn_cur[3][0:1, 0:P])
        outproj(2)
        emit_rs(2, RS_SPLIT_LAST)
        for i in deferred_stores:
            gp.dma_start(
                out.ap()[sum(rs_rows[:i]):sum(rs_rows[:i + 1]), :],
                rs_out[i][:])

    return nc


_PROGRAM = None


def _get_program():
    global _PROGRAM
    if _PROGRAM is None:
        from contextlib import ExitStack
        import concourse.tile as tile
        from concourse import bacc, mybir

        nc = bacc.Bacc("TRN2", target_bir_lowering=False, debug=False,
                       num_devices=NCORES)
        _emit(nc, tile, mybir, ExitStack)
        nc.compile()
        _PROGRAM = nc
    return _PROGRAM


def _bf16(a):
    from ml_dtypes import bfloat16
    return np.asarray(a, np.float32).astype(bfloat16)


def _perm_eo(w):
    """Per 128-row block: rows -> [even rows (64), odd rows (64)]."""
    n = w.shape[0] // P
    w = w.reshape(n, 64, 2, w.shape[-1])
    return np.concatenate([w[:, :, 0, :], w[:, :, 1, :]], axis=1).reshape(
        n * P, -1)


def make_in_maps(x, wq, wk, wv, wo, freqs_cos, freqs_sin):
    x = np.asarray(x, np.float32)
    cos = np.asarray(freqs_cos, np.float32)
    sin = np.asarray(freqs_sin, np.float32)

    cosT = cos.T                      # [64, T]
    sinT = sin.T
    ak = _bf16(np.ascontiguousarray(np.vstack([cosT, cosT])))
    bk = _bf16(np.ascontiguousarray(np.vstack([-sinT, sinT])))
    aq = _bf16(SCALE * np.vstack([cosT, cosT]))
    bq = _bf16(SCALE * np.vstack([-sinT, sinT]))

    xt4s = []
    for b in range(B):
        xt = _bf16(x[b]).T            # [C, T]
        xt4 = np.ascontiguousarray(
            xt.reshape(C, NCH, 512).transpose(1, 0, 2)).reshape(NCH * C, 512)
        xt4s.append(xt4)

    # device constants: causal mask (scoresT layout), identity, ones
    tri = np.where(np.arange(P)[None, :] >= np.arange(P)[:, None],
                   np.float32(0.0), np.float32(NEG)).astype(np.float32)
    cstb = np.zeros((P, 2 * P + 2), np.float32)
    cstb[:, 0:P] = np.eye(P, dtype=np.float32)
    cstb[:, P:] = 1.0
    cstb = _bf16(cstb)

    in_maps = []
    for core in range(NCORES):
        b, g = core // 4, core % 4
        wq_g = _perm_eo(np.asarray(wq[g * HL * D:(g + 1) * HL * D], np.float32))
        wk_g = _perm_eo(np.asarray(wk[g * D:(g + 1) * D], np.float32))
        wv_g = np.asarray(wv[g * D:(g + 1) * D], np.float32)
        wqkv_g = _bf16(np.ascontiguousarray(
            np.concatenate([wq_g.T, wk_g.T, wv_g.T], axis=1)))
        wot_g = _bf16(np.ascontiguousarray(
            np.asarray(wo, np.float32)[:, g * HL * D:(g + 1) * HL * D].T))
        in_maps.append({
            "xt4": xt4s[b],
            "wqkv": wqkv_g,
            "wot": wot_g,
            "raq": aq, "rbq": bq, "rak": ak, "rbk": bk,
            "cstf": tri, "cstb": cstb,
        })
    return in_maps


def kernel(x, wq, wk, wv, wo, freqs_cos, freqs_sin, mask=None):
    from concourse.bass_utils import run_bass_kernel_spmd

    nc = _get_program()
    in_maps = make_in_maps(x, wq, wk, wv, wo, freqs_cos, freqs_sin)
    res = run_bass_kernel_spmd(nc, in_maps, core_ids=list(range(NCORES)))
    outp = np.empty((B, T, C), np.float32)
    # piece row blocks, in RS emission order (groups 0,1,3,2):
    # (piece_row_start, nrows, global query base for core stripe 0)
    blocks = [(0, 64, 0), (64, 64, 256),        # g0 halves
              (128, 64, 512), (192, 64, 768),   # g1 halves
              (256, 64, 1536), (320, 64, 1792),  # g3 halves
              (384, 128, 1024)]                  # g2 single RS
    for b in range(B):
        for r in range(4):
            piece = np.asarray(res.results[4 * b + r]["out"],
                               dtype=np.float32)  # [512, C]
            for prow, nrows, qbase in blocks:
                dst = qbase + nrows * r
                outp[b, dst:dst + nrows] = piece[prow:prow + nrows]
    return outp
